# revision 26
# baseline (speedup 1.0000x reference)
"""Trainium2 Bass kernel for per-channel local attention (AttentionConv).

Reference computation (per batch element b):
    q = Wq @ x          [O, L]
    k = Wk @ xp         [O, L+6]   (xp = x padded by 3 on both ends of L)
    v = Wv @ xp
    t_j = q * (k[:, j:j+L] + rel[:, j])     j = 0..6
    out = sum_j exp(t_j) * v[:, j:j+L] / sum_j exp(t_j)

Sharding: data-parallel over batch. B=8 batch elements -> 8 NeuronCores,
one full batch element per core; no cross-core communication.

Engine mapping (per core):
  PE:   k/q/v projections (bf16), windowed sums via identity-matmul PSUM accum
  ACT:  PSUM->SBUF casts (bf16), exp
  DVE:  kw_j = k_j + rel_j (tensor_scalar 4x, even j), t_j = kw_j * q,
        ev_j = e_j * v_j (TT 2x), reciprocal, final multiply
  GpSimd: kw_j for odd j (no alignment constraint on the Q7 path),
        v odd-shift copy, weight/const input DMAs (SWDGE queue)

Emission is software-pipelined: projections of tile ot+1 are emitted before
the reduce of tile ot so the PE stream never stalls on the elementwise
stages; the elementwise work runs in L-halves with double-buffered planes.
"""

import sys

try:
    import concourse  # noqa: F401
except ImportError:  # grading container has the repo at this fixed path
    sys.path.insert(0, "/opt/trn_rl_repo")

from contextlib import ExitStack

import ml_dtypes
import numpy as np

import concourse.bass as bass
import concourse.mybir as mybir
import concourse.tile as tile
from concourse import bacc

BF16 = ml_dtypes.bfloat16

# Problem shape (hardcoded; harness always calls with these shapes)
B = 8
CIN = 512
COUT = 512
L = 2048
KW = 7
PAD = 3
LP = L + 2 * PAD  # 2054
P = 128
NC_TILES_O = COUT // P  # 4 output-channel tiles
NC_TILES_K = CIN // P   # 4 contraction tiles
NCH = 4                 # 512-wide L chunks
CH = 512
H = 1024                # elementwise pipeline block width

F32 = mybir.dt.float32
BF = mybir.dt.bfloat16


def build_nc():
    nc = bacc.Bacc("TRN2", target_bir_lowering=False, debug=False)

    xp_d = nc.dram_tensor("xp", [CIN, LP], BF, kind="ExternalInput")
    wq_d = nc.dram_tensor("wq", [CIN, COUT], BF, kind="ExternalInput")
    wk_d = nc.dram_tensor("wk", [CIN, COUT], BF, kind="ExternalInput")
    wv_d = nc.dram_tensor("wv", [CIN, COUT], BF, kind="ExternalInput")
    rel_d = nc.dram_tensor("rel", [COUT, 8], F32, kind="ExternalInput")
    id_d = nc.dram_tensor("ident", [P, P], BF, kind="ExternalInput")
    out_d = nc.dram_tensor("out", [COUT, L], F32, kind="ExternalOutput")

    op_add = mybir.AluOpType.add

    with tile.TileContext(nc) as tc, ExitStack() as ctx:
        singles = ctx.enter_context(tc.tile_pool(name="singles", bufs=1))
        qkv_pool = ctx.enter_context(tc.tile_pool(name="qkv", bufs=2))
        planes = ctx.enter_context(tc.tile_pool(name="planes", bufs=2))
        outp = ctx.enter_context(tc.tile_pool(name="outp", bufs=2))
        smalls = ctx.enter_context(tc.tile_pool(name="smalls", bufs=2))
        psum_big = ctx.enter_context(tc.tile_pool(name="psum_big", bufs=2, space="PSUM"))
        psum_red = ctx.enter_context(tc.tile_pool(name="psum_red", bufs=2, space="PSUM"))

        # --- persistent inputs; spread DMAs over queues (xp via SP HWDGE,
        # weights/consts via GpSimd SWDGE) ordered so the k projection's
        # inputs arrive first ---
        w_sb = {"k": [], "q": [], "v": []}
        xp_sb = []
        for kt in range(NC_TILES_K):
            t_ = singles.tile([P, LP], BF, tag=f"xp{kt}")
            xp_sb.append(t_)
        for kt in range(NC_TILES_K):
            wt = singles.tile([P, COUT], BF, tag=f"wk{kt}")
            nc.gpsimd.dma_start(out=wt[:, :], in_=wk_d[kt * P:(kt + 1) * P, :])
            w_sb["k"].append(wt)
            nc.sync.dma_start(out=xp_sb[kt][:, 0:1027],
                              in_=xp_d[kt * P:(kt + 1) * P, 0:1027])
        ident = singles.tile([P, P], BF, tag="ident")
        nc.gpsimd.dma_start(out=ident[:, :], in_=id_d[:, :])
        rel_sb = []
        for ot in range(NC_TILES_O):
            t_ = singles.tile([P, 8], F32, tag=f"rel{ot}")
            nc.gpsimd.dma_start(out=t_[:, :], in_=rel_d[ot * P:(ot + 1) * P, :])
            rel_sb.append(t_)
        for kt in range(NC_TILES_K):
            wt = singles.tile([P, COUT], BF, tag=f"wq{kt}")
            nc.gpsimd.dma_start(out=wt[:, :], in_=wq_d[kt * P:(kt + 1) * P, :])
            w_sb["q"].append(wt)
            nc.sync.dma_start(out=xp_sb[kt][:, 1027:LP],
                              in_=xp_d[kt * P:(kt + 1) * P, 1027:LP])
        for kt in range(NC_TILES_K):
            wt = singles.tile([P, COUT], BF, tag=f"wv{kt}")
            nc.gpsimd.dma_start(out=wt[:, :], in_=wv_d[kt * P:(kt + 1) * P, :])
            w_sb["v"].append(wt)

        state = {}

        def emit_projections(ot):
            q_sb = qkv_pool.tile([P, L], BF, tag="q")
            k_sb = qkv_pool.tile([P, 2056], BF, tag="k")
            v_sb = qkv_pool.tile([P, 2056], BF, tag="v")
            v_odd = qkv_pool.tile([P, 2056], BF, tag="vo")
            for name, dst, width in (("k", k_sb, LP), ("q", q_sb, L), ("v", v_sb, LP)):
                off = PAD if name == "q" else 0  # q is over unpadded x
                for hb in range(2):
                    ps = psum_big.tile([P, 2, CH], F32, tag="big")
                    for n in (2 * hb, 2 * hb + 1):
                        for kt in range(NC_TILES_K):
                            nc.tensor.matmul(
                                ps[:, n - 2 * hb, :],
                                w_sb[name][kt][:, ot * P:(ot + 1) * P],
                                xp_sb[kt][:, off + n * CH:off + (n + 1) * CH],
                                start=(kt == 0),
                                stop=(kt == NC_TILES_K - 1),
                            )
                    nc.scalar.copy(
                        out=dst[:, 2 * hb * CH:(2 * hb + 2) * CH].rearrange(
                            "p (n c) -> p n c", n=2),
                        in_=ps[:, :, :],
                    )
                if width > L:  # ragged 6-col tail for k and v
                    rg = psum_red.tile([P, CH], F32, tag="den")
                    for kt in range(NC_TILES_K):
                        nc.tensor.matmul(
                            rg[:, 0:LP - L],
                            w_sb[name][kt][:, ot * P:(ot + 1) * P],
                            xp_sb[kt][:, L:LP],
                            start=(kt == 0),
                            stop=(kt == NC_TILES_K - 1),
                        )
                    nc.scalar.copy(out=dst[:, L:LP], in_=rg[:, 0:LP - L])
            # odd-aligned copy so bf16 packed (2x) reads stay 4B-aligned in ev
            nc.gpsimd.tensor_copy(out=v_odd[:, 0:LP - 1], in_=v_sb[:, 1:LP])
            state[ot] = (q_sb, k_sb, v_sb, v_odd)

        def emit_block(ot):
            q_sb, k_sb, v_sb, v_odd = state[ot]
            out_sb = outp.tile([P, L], F32, tag="o")
            for h in range(L // H):
                h0 = h * H
                kw_buf = planes.tile([P, KW, H], BF, tag="kw")
                for j in range(KW):
                    eng = nc.vector if j % 2 == 0 else nc.gpsimd
                    eng.tensor_scalar(
                        kw_buf[:, j, :], k_sb[:, h0 + j:h0 + j + H],
                        rel_sb[ot][:, j:j + 1], None, op_add,
                    )
                t_buf = planes.tile([P, KW, H], BF, tag="t")
                for j in range(KW):
                    nc.vector.tensor_mul(
                        t_buf[:, j, :], kw_buf[:, j, :], q_sb[:, h0:h0 + H])
                e_buf = planes.tile([P, KW, H], BF, tag="e")
                for j in range(KW):
                    nc.scalar.activation(
                        out=e_buf[:, j, :], in_=t_buf[:, j, :],
                        func=mybir.ActivationFunctionType.Exp,
                    )
                ev_buf = kw_buf
                for j in range(KW):
                    vsrc = (v_sb[:, h0 + j:h0 + j + H] if j % 2 == 0
                            else v_odd[:, h0 + j - 1:h0 + j - 1 + H])
                    nc.vector.tensor_mul(ev_buf[:, j, :], e_buf[:, j, :], vsrc)

                for nn in range(H // CH):
                    n = (h0 // CH) + nn
                    c0 = nn * CH
                    den_ps = psum_red.tile([P, CH], F32, tag="den")
                    num_ps = psum_red.tile([P, CH], F32, tag="num")
                    for j in range(KW):
                        nc.tensor.matmul(
                            den_ps[:, :], ident[:, :],
                            e_buf[:, j, c0:c0 + CH],
                            start=(j == 0), stop=(j == KW - 1),
                        )
                    for j in range(KW):
                        nc.tensor.matmul(
                            num_ps[:, :], ident[:, :],
                            ev_buf[:, j, c0:c0 + CH],
                            start=(j == 0), stop=(j == KW - 1),
                        )
                    r_sb = smalls.tile([P, CH], F32, tag="r")
                    nc.vector.reciprocal_approx_fast(out=r_sb[:, :], in_=den_ps[:, :])
                    nc.vector.tensor_mul(
                        out_sb[:, n * CH:(n + 1) * CH], num_ps[:, :], r_sb[:, :])
                nc.sync.dma_start(out=out_d[ot * P:(ot + 1) * P, h0:h0 + H],
                                  in_=out_sb[:, h0:h0 + H])
            del state[ot]

        # software-pipelined emission: projections run one tile ahead
        emit_projections(0)
        for ot in range(NC_TILES_O):
            if ot + 1 < NC_TILES_O:
                emit_projections(ot + 1)
            emit_block(ot)

    nc.compile()
    return nc


def make_in_maps(x, Wq, Wk, Wv, rel_h, rel_w):
    x = np.asarray(x, dtype=np.float32)
    xp = np.pad(x, ((0, 0), (0, 0), (PAD, PAD)))
    rel = np.concatenate([np.asarray(rel_h), np.asarray(rel_w)], axis=0)[:, 0, :]
    rel8 = np.zeros((COUT, 8), np.float32)
    rel8[:, :KW] = rel
    wq = np.ascontiguousarray(np.asarray(Wq, dtype=np.float32).T).astype(BF16)
    wk = np.ascontiguousarray(np.asarray(Wk, dtype=np.float32).T).astype(BF16)
    wv = np.ascontiguousarray(np.asarray(Wv, dtype=np.float32).T).astype(BF16)
    ident = np.eye(P, dtype=BF16)
    return [
        {
            "xp": np.ascontiguousarray(xp[b]).astype(BF16),
            "wq": wq, "wk": wk, "wv": wv,
            "rel": rel8, "ident": ident,
        }
        for b in range(B)
    ]


_NC_CACHE = None


def kernel(x, Wq, Wk, Wv, rel_h, rel_w):
    global _NC_CACHE
    from concourse.bass_utils import run_bass_kernel_spmd

    if _NC_CACHE is None:
        _NC_CACHE = build_nc()
    nc = _NC_CACHE
    in_maps = make_in_maps(x, Wq, Wk, Wv, rel_h, rel_w)
    res = run_bass_kernel_spmd(nc, in_maps, core_ids=list(range(B)))
    out = np.stack([np.asarray(res.results[b]["out"]) for b in range(B)])
    return out.astype(np.float32)


# revision 52
# speedup vs baseline: 1.0466x; 1.0466x over previous
"""Trainium2 Bass kernel for per-channel local attention (AttentionConv).

Reference computation (per batch element b):
    q = Wq @ x          [O, L]
    k = Wk @ xp         [O, L+6]   (xp = x padded by 3 on both ends of L)
    v = Wv @ xp
    t_j = q * (k[:, j:j+L] + rel[:, j])     j = 0..6
    out = sum_j exp(t_j) * v[:, j:j+L] / sum_j exp(t_j)

Sharding: data-parallel over batch. B=8 batch elements -> 8 NeuronCores,
one full batch element per core; no cross-core communication.

Engine mapping (per core):
  PE:   k/q/v projections (bf16), windowed sums via identity-matmul PSUM accum
  ACT:  PSUM->SBUF casts (bf16), exp
  DVE:  kw_j = k_j + rel_j (tensor_scalar 4x, even j), t_j = kw_j * q,
        ev_j = e_j * v_j (TT 2x), reciprocal, final multiply
  GpSimd: kw_j for odd j (no alignment constraint on the Q7 path),
        v odd-shift copy, weight/const input DMAs (SWDGE queue)

Emission is software-pipelined: projections of tile ot+1 are emitted before
the reduce of tile ot so the PE stream never stalls on the elementwise
stages; the elementwise work runs in L-halves with double-buffered planes.
"""

import sys

try:
    import concourse  # noqa: F401
except ImportError:  # grading container has the repo at this fixed path
    sys.path.insert(0, "/opt/trn_rl_repo")

from contextlib import ExitStack

import ml_dtypes
import numpy as np

import concourse.bass as bass
import concourse.mybir as mybir
import concourse.tile as tile
from concourse import bacc

BF16 = ml_dtypes.bfloat16

# Problem shape (hardcoded; harness always calls with these shapes)
B = 8
CIN = 512
COUT = 512
L = 2048
KW = 7
PAD = 3
LP = L + 2 * PAD  # 2054
P = 128
NC_TILES_O = COUT // P  # 4 output-channel tiles
NC_TILES_K = CIN // P   # 4 contraction tiles
NCH = 4                 # 512-wide L chunks
CH = 512
H = 1024                # elementwise pipeline block width

F32 = mybir.dt.float32
BF = mybir.dt.bfloat16


def build_nc():
    nc = bacc.Bacc("TRN2", target_bir_lowering=False, debug=False)

    xp_d = nc.dram_tensor("xp", [CIN, LP], BF, kind="ExternalInput")
    wq_d = nc.dram_tensor("wq", [CIN, COUT], BF, kind="ExternalInput")
    wk_d = nc.dram_tensor("wk", [CIN, COUT], BF, kind="ExternalInput")
    wv_d = nc.dram_tensor("wv", [CIN, COUT], BF, kind="ExternalInput")
    rel_d = nc.dram_tensor("rel", [COUT, 8], F32, kind="ExternalInput")
    id_d = nc.dram_tensor("ident", [P, P], BF, kind="ExternalInput")
    out_d = nc.dram_tensor("out", [COUT, L], F32, kind="ExternalOutput")

    op_add = mybir.AluOpType.add

    with tile.TileContext(nc) as tc, ExitStack() as ctx:
        singles = ctx.enter_context(tc.tile_pool(name="singles", bufs=1))
        qkv_pool = ctx.enter_context(tc.tile_pool(name="qkv", bufs=2))
        planes = ctx.enter_context(tc.tile_pool(name="planes", bufs=3))
        outp = ctx.enter_context(tc.tile_pool(name="outp", bufs=2))
        smalls = ctx.enter_context(tc.tile_pool(name="smalls", bufs=2))
        psum_big = ctx.enter_context(tc.tile_pool(name="psum_big", bufs=2, space="PSUM"))
        psum_red = ctx.enter_context(tc.tile_pool(name="psum_red", bufs=2, space="PSUM"))

        # --- persistent inputs; spread DMAs over queues (xp via SP HWDGE,
        # wk via ACT HWDGE, remaining weights/consts via GpSimd SWDGE)
        # ordered so the k projection's inputs arrive first ---
        w_sb = {"k": [], "q": [], "v": []}
        xp_sb = []
        for kt in range(NC_TILES_K):
            t_ = singles.tile([P, LP], BF, tag=f"xp{kt}")
            xp_sb.append(t_)
        for kt in range(NC_TILES_K):
            wt = singles.tile([P, COUT], BF, tag=f"wk{kt}")
            nc.scalar.dma_start(out=wt[:, :], in_=wk_d[kt * P:(kt + 1) * P, :])
            w_sb["k"].append(wt)
            nc.sync.dma_start(out=xp_sb[kt][:, 0:515],
                              in_=xp_d[kt * P:(kt + 1) * P, 0:515])
        for kt in range(NC_TILES_K):
            nc.sync.dma_start(out=xp_sb[kt][:, 515:1027],
                              in_=xp_d[kt * P:(kt + 1) * P, 515:1027])
        ident = singles.tile([P, P], BF, tag="ident")
        nc.gpsimd.dma_start(out=ident[:, :], in_=id_d[:, :])
        rel_sb = []
        for ot in range(NC_TILES_O):
            t_ = singles.tile([P, 8], F32, tag=f"rel{ot}")
            nc.gpsimd.dma_start(out=t_[:, :], in_=rel_d[ot * P:(ot + 1) * P, :])
            rel_sb.append(t_)
        for kt in range(NC_TILES_K):
            wt = singles.tile([P, COUT], BF, tag=f"wq{kt}")
            nc.gpsimd.dma_start(out=wt[:, :], in_=wq_d[kt * P:(kt + 1) * P, :])
            w_sb["q"].append(wt)
            nc.sync.dma_start(out=xp_sb[kt][:, 1027:LP],
                              in_=xp_d[kt * P:(kt + 1) * P, 1027:LP])
        for kt in range(NC_TILES_K):
            wt = singles.tile([P, COUT], BF, tag=f"wv{kt}")
            nc.gpsimd.dma_start(out=wt[:, :], in_=wv_d[kt * P:(kt + 1) * P, :])
            w_sb["v"].append(wt)

        state = {}

        def emit_projections(ot):
            q_sb = qkv_pool.tile([P, L], BF, tag="q")
            k_sb = qkv_pool.tile([P, 2056], BF, tag="k")
            v_sb = qkv_pool.tile([P, 2056], BF, tag="v")
            v_odd = qkv_pool.tile([P, 2056], BF, tag="vo")
            kw_bufs = []

            def _after(name):
                if name != "k":
                    return
                # kw_j = k_j + rel_j as soon as k lands (DVE/GpSimd lead-in)
                for h in range(L // H):
                    h0 = h * H
                    kw_buf = planes.tile([P, KW, H], BF, tag="kw")
                    for j in range(KW):
                        eng = nc.vector if j % 2 == 0 else nc.gpsimd
                        eng.tensor_scalar(
                            kw_buf[:, j, :], k_sb[:, h0 + j:h0 + j + H],
                            rel_sb[ot][:, j:j + 1], None, op_add,
                        )
                    kw_bufs.append(kw_buf)

            for name, dst, width in (("k", k_sb, LP), ("q", q_sb, L), ("v", v_sb, LP)):
                off = PAD if name == "q" else 0  # q is over unpadded x
                for hb in range(2):
                    ps = psum_big.tile([P, 2, CH], F32, tag="big")
                    for n in (2 * hb, 2 * hb + 1):
                        for kt in range(NC_TILES_K):
                            nc.tensor.matmul(
                                ps[:, n - 2 * hb, :],
                                w_sb[name][kt][:, ot * P:(ot + 1) * P],
                                xp_sb[kt][:, off + n * CH:off + (n + 1) * CH],
                                start=(kt == 0),
                                stop=(kt == NC_TILES_K - 1),
                            )
                    nc.scalar.copy(
                        out=dst[:, 2 * hb * CH:(2 * hb + 2) * CH].rearrange(
                            "p (n c) -> p n c", n=2),
                        in_=ps[:, :, :],
                    )
                if width > L:  # ragged 6-col tail for k and v
                    rg = psum_red.tile([P, CH], F32, tag="den")
                    for kt in range(NC_TILES_K):
                        nc.tensor.matmul(
                            rg[:, 0:LP - L],
                            w_sb[name][kt][:, ot * P:(ot + 1) * P],
                            xp_sb[kt][:, L:LP],
                            start=(kt == 0),
                            stop=(kt == NC_TILES_K - 1),
                        )
                    nc.scalar.copy(out=dst[:, L:LP], in_=rg[:, 0:LP - L])
                _after(name)
            # odd-aligned copy so bf16 packed (2x) reads stay 4B-aligned in ev
            nc.gpsimd.tensor_copy(out=v_odd[:, 0:LP - 1], in_=v_sb[:, 1:LP])
            state[ot] = (q_sb, k_sb, v_sb, v_odd, kw_bufs)

        def emit_block(ot):
            q_sb, k_sb, v_sb, v_odd, kw_bufs = state[ot]
            out_sb = outp.tile([P, L], F32, tag="o")
            for h in range(L // H):
                h0 = h * H
                kw_buf = kw_bufs[h]
                t_buf = planes.tile([P, KW, H], BF, tag="t")
                for j in range(KW):
                    eng = nc.gpsimd if j == 6 else nc.vector
                    eng.tensor_mul(
                        t_buf[:, j, :], kw_buf[:, j, :], q_sb[:, h0:h0 + H])
                e_buf = planes.tile([P, KW, H], BF, tag="e")
                for j in range(KW):
                    nc.scalar.activation(
                        out=e_buf[:, j, :], in_=t_buf[:, j, :],
                        func=mybir.ActivationFunctionType.Exp,
                    )
                ev_buf = kw_buf
                for j in range(KW):
                    vsrc = (v_sb[:, h0 + j:h0 + j + H] if j % 2 == 0
                            else v_odd[:, h0 + j - 1:h0 + j - 1 + H])
                    nc.vector.tensor_mul(ev_buf[:, j, :], e_buf[:, j, :], vsrc)

                dens = []
                rs = []
                for nn in range(H // CH):
                    c0 = nn * CH
                    den_ps = psum_red.tile([P, CH], F32, tag="den")
                    for j in range(KW):
                        nc.tensor.matmul(
                            den_ps[:, :], ident[:, :],
                            e_buf[:, j, c0:c0 + CH],
                            start=(j == 0), stop=(j == KW - 1),
                        )
                    r_sb = smalls.tile([P, CH], F32, tag="r")
                    nc.vector.reciprocal_approx_fast(out=r_sb[:, :], in_=den_ps[:, :])
                    dens.append(den_ps)
                    rs.append(r_sb)
                for nn in range(H // CH):
                    n = (h0 // CH) + nn
                    c0 = nn * CH
                    num_ps = psum_red.tile([P, CH], F32, tag="num")
                    for j in range(KW):
                        nc.tensor.matmul(
                            num_ps[:, :], ident[:, :],
                            ev_buf[:, j, c0:c0 + CH],
                            start=(j == 0), stop=(j == KW - 1),
                        )
                    nc.vector.tensor_mul(
                        out_sb[:, n * CH:(n + 1) * CH], num_ps[:, :], rs[nn][:, :])
                nc.sync.dma_start(out=out_d[ot * P:(ot + 1) * P, h0:h0 + H],
                                  in_=out_sb[:, h0:h0 + H])
            del state[ot]

        # software-pipelined emission: projections run one tile ahead
        emit_projections(0)
        for ot in range(NC_TILES_O):
            if ot + 1 < NC_TILES_O:
                emit_projections(ot + 1)
            emit_block(ot)

    nc.compile()
    return nc


def make_in_maps(x, Wq, Wk, Wv, rel_h, rel_w):
    x = np.asarray(x, dtype=np.float32)
    xp = np.pad(x, ((0, 0), (0, 0), (PAD, PAD)))
    rel = np.concatenate([np.asarray(rel_h), np.asarray(rel_w)], axis=0)[:, 0, :]
    rel8 = np.zeros((COUT, 8), np.float32)
    rel8[:, :KW] = rel
    wq = np.ascontiguousarray(np.asarray(Wq, dtype=np.float32).T).astype(BF16)
    wk = np.ascontiguousarray(np.asarray(Wk, dtype=np.float32).T).astype(BF16)
    wv = np.ascontiguousarray(np.asarray(Wv, dtype=np.float32).T).astype(BF16)
    ident = np.eye(P, dtype=BF16)
    return [
        {
            "xp": np.ascontiguousarray(xp[b]).astype(BF16),
            "wq": wq, "wk": wk, "wv": wv,
            "rel": rel8, "ident": ident,
        }
        for b in range(B)
    ]


_NC_CACHE = None


def kernel(x, Wq, Wk, Wv, rel_h, rel_w):
    global _NC_CACHE
    from concourse.bass_utils import run_bass_kernel_spmd

    if _NC_CACHE is None:
        _NC_CACHE = build_nc()
    nc = _NC_CACHE
    in_maps = make_in_maps(x, Wq, Wk, Wv, rel_h, rel_w)
    res = run_bass_kernel_spmd(nc, in_maps, core_ids=list(range(B)))
    out = np.stack([np.asarray(res.results[b]["out"]) for b in range(B)])
    return out.astype(np.float32)
